# revision 39
# baseline (speedup 1.0000x reference)
"""Trainium2 Bass kernel for nn_GAT_42786464203341.

8-way tensor parallel (Megatron-style) over one trn2 chip:
  - The GAT edges are block-diagonal fully-connected per sample, so message
    passing is dense per-sample attention with scores leaky(el[i] + er[j]),
    softmaxed over source i.
  - Activations are feature-major (x^T: [D, nodes]); contraction on
    partitions feeds the PE array directly.
  - LayerNorm is folded into the following matmul: with Wg = diag(g)W,
    y = r*(Wg^T x) + m*f1 + std*f2 scaled by r, where f1 = -colsum(Wg),
    f2 = W^T b_ln.  The rank-2 term is accumulated into the same PSUM
    group (lhsT/rhs = [m_row; std_row] x [f1; f2]); the *r scale happens
    at the PSUM->SBUF copy.  LN stats come from vector-engine partial-sum
    trees + one ones-vector matmul each for sum and sum-of-squares.
  - Attention is head-parallel (2 heads/core); W_proj row-sharded ->
    partial [D, nodes] -> AllReduce.  FFN column/row sharded -> AllReduce.
    Head is vocab-sharded; host concatenates the 8 logits slices.
  - AllReduces are split per batch (12 total) and software-pipelined:
    AR(batch 0) overlaps compute of batch 1 and vice versa.
  - Weight / staging / refresh DMAs are batched into single descriptors
    (host pre-permutes weights into [128, ktiles, cols] layout).
"""

import time
from contextlib import ExitStack

import ml_dtypes
import numpy as np

import concourse.bass as bass
import concourse.tile as tile
from concourse import bacc, mybir
from concourse.masks import make_identity

F32 = mybir.dt.float32
F32R = mybir.dt.float32r
F16 = mybir.dt.float16
BF16 = mybir.dt.bfloat16

B, T, NOBJ = 2, 265, 9
D, H, DH = 1536, 16, 96
V, PV, L, FF = 8192, 512, 3, 6144
N = B * T          # 530
NC = 8             # cores
HPC = H // NC      # heads per core
FFL = FF // NC     # 768
VL = V // NC       # 1024
NCH = T + 1        # 266 (col 265 of each chunk is zero padding)
NP = B * NCH       # 532
KD = D // 128      # 12
KF = FFL // 128    # 6
KV = VL // 128     # 8
MT = [(0, 128), (128, 128), (256, 10)]   # node tiles per batch (start, size)
MT_REAL = [128, 128, 9]                  # non-pad rows per node tile
EPS = 1e-5

# gb blob column offsets: bout8, bproj8 (KD each), bfc (KF)
GB_BOUT, GB_BPROJ, GB_BFC = 0, KD, 2 * KD
GB_W = 2 * KD + KF

_CACHE = {}


# --------------------------------------------------------------------------
# host-side input prep
# --------------------------------------------------------------------------

def _block_diag_edges_np():
    base = np.arange(T)
    src = np.concatenate([g * T + np.repeat(base, T) for g in range(B)])
    dst = np.concatenate([g * T + np.tile(base, T) for g in range(B)])
    return src.astype(np.int64), dst.astype(np.int64)


def _perm_k(w, ktiles):
    """[.., K*128, cols] -> [.., 128, ktiles, cols] (partition-major)."""
    s = w.shape
    return np.ascontiguousarray(
        w.reshape(*s[:-2], ktiles, 128, s[-1]).swapaxes(-3, -2)
    )


def _host_inputs(inp, ffn_bf16=True):
    f32 = np.float32
    bf16 = ml_dtypes.bfloat16
    objs_e = np.asarray(inp["obj_emb_w"])[np.asarray(inp["objs"])]
    pe = np.asarray(inp["poss_emb_w"])[np.asarray(inp["poss"])]
    nfeat = np.concatenate([objs_e, pe[:, :NOBJ], pe[:, NOBJ:]], axis=-1)
    z = np.asarray(inp["tok_emb"])[np.asarray(inp["z_indices"])]
    x0 = np.concatenate([nfeat, z], axis=1) + np.asarray(inp["pos_emb"])[:, :T]
    x0 = x0.reshape(N, D).astype(f32)

    x0t = np.zeros((D, NP), f32)
    for b in range(B):
        x0t[:, b * NCH:b * NCH + T] = x0[b * T:(b + 1) * T].T
    x0tp = _perm_k(x0t, KD)                       # [128, KD, NP]

    W_attn = np.asarray(inp["W_attn"], f32)
    a_l = np.asarray(inp["a_l"], f32)
    a_r = np.asarray(inp["a_r"], f32)
    W_proj = np.asarray(inp["W_proj"], f32)
    W_fc = np.asarray(inp["W_fc"], f32)
    W_out = np.asarray(inp["W_out"], f32)
    head_w = np.asarray(inp["head_w"], f32)
    g1 = np.asarray(inp["ln1_g"], f32)            # [L, D]
    b1 = np.asarray(inp["ln1_b"], f32)
    g2 = np.asarray(inp["ln2_g"], f32)
    b2 = np.asarray(inp["ln2_b"], f32)
    gf = np.asarray(inp["lnf_g"], f32)            # [D]
    bf = np.asarray(inp["lnf_b"], f32)

    def cols(vec, k_tiles):  # [3, D'] -> [3, 128, k_tiles]
        v = np.asarray(vec, f32)
        return np.transpose(v.reshape(3, k_tiles, 128), (0, 2, 1)).copy()

    bfc_all = cols(inp["b_fc"], KF * NC)
    bout8 = cols(np.asarray(inp["b_out"], f32) / NC, KD)
    bproj8 = cols(np.asarray(inp["b_proj"], f32) / NC, KD)

    # LN2-folded FC weights and fixups (full, sliced per core below)
    wfc_g_full = W_fc * g2[:, :, None]            # [L, D, FF]
    f1_fc_full = -wfc_g_full.sum(axis=1)          # [L, FF]
    f2_fc_full = np.einsum("ld,ldo->lo", b2, W_fc)
    # LNF-folded head weights and fixups
    whead_g = head_w * gf[:, None]                # [D, V]
    f1_hd_full = -whead_g.sum(axis=0)             # [V]
    f2_hd_full = bf @ head_w                      # [V]

    wdt = bf16 if ffn_bf16 else f32
    maps = []
    for c in range(NC):
        h0 = c * HPC
        wattn = np.zeros((L, D, 256), f32)
        for j in range(HPC):
            hg = h0 + j
            blk = W_attn[:, :, hg * DH:(hg + 1) * DH]         # [3, D, DH]
            wattn[:, :, j * DH:(j + 1) * DH] = blk
            # el/er are linear in h: fold (W_attn-block @ a) into one column
            wattn[:, :, 192 + j] = np.matmul(blk, a_l[:, hg, :, None])[..., 0]
            wattn[:, :, 194 + j] = np.matmul(blk, a_r[:, hg, :, None])[..., 0]
        # LN1 fold for the attention matmul
        f2_at = np.einsum("ld,ldo->lo", b1, wattn)            # [L, 256]
        wattn_g = wattn * g1[:, :, None]
        f1_at = -wattn_g.sum(axis=1)                          # [L, 256]
        f_attn = np.stack([f1_at, f2_at], axis=1)             # [L, 2, 256]

        wproj = np.stack(
            [W_proj[:, (h0 + j) * DH:(h0 + j + 1) * DH, :] for j in range(HPC)],
            axis=2,
        )                                          # [L, DH, HPC, D]
        gb = np.concatenate(
            [bout8, bproj8, bfc_all[:, :, c * KF:(c + 1) * KF]],
            axis=2,
        ).copy()                                   # [L, 128, GB_W]
        fsl = slice(c * FFL, (c + 1) * FFL)
        vsl = slice(c * VL, (c + 1) * VL)
        maps.append({
            "x0t": x0tp,
            "wattn": _perm_k(wattn_g, KD).astype(f32),   # f32r on device
            "fattn": np.ascontiguousarray(f_attn),
            "wproj": np.ascontiguousarray(wproj).astype(bf16),
            "wfc": _perm_k(
                np.ascontiguousarray(wfc_g_full[:, :, fsl]), KD
            ).astype(wdt),                         # [L, 128, KD, FFL]
            "ffc": np.ascontiguousarray(
                np.stack([f1_fc_full[:, fsl], f2_fc_full[:, fsl]], axis=1)
            ),                                     # [L, 2, FFL]
            "wout": _perm_k(
                np.ascontiguousarray(W_out[:, fsl, :]), KF
            ).astype(wdt),                         # [L, 128, KF, D]
            "whead": _perm_k(
                np.ascontiguousarray(whead_g[:, vsl]), KD
            ).astype(bf16),                        # [128, KD, VL]
            "fhead": np.ascontiguousarray(
                np.stack([f1_hd_full[vsl], f2_hd_full[vsl]], axis=0)
            ),                                     # [2, VL]
            "ones_col": np.ones((128, 1), f32),
            "ones_colh": np.ones((128, 1), bf16),
            "ones_row": np.ones((1, 128), f32),
            "gb": gb,
        })
    return maps


# --------------------------------------------------------------------------
# device program
# --------------------------------------------------------------------------

def _build_nc(reps=1, use_cc=True, ffn_bf16=True):
    nc = bacc.Bacc("TRN2", target_bir_lowering=False, debug=False, num_devices=NC)

    d_x0t = nc.declare_dram_parameter("x0t", [128, KD, NP], F32R, isOutput=False)
    d_wattn = nc.declare_dram_parameter("wattn", [L, 128, KD, 256], F32R, isOutput=False)
    d_fattn = nc.declare_dram_parameter("fattn", [L, 2, 256], F32R, isOutput=False)
    d_wproj = nc.declare_dram_parameter("wproj", [L, DH, HPC, D], BF16, isOutput=False)
    WDT = BF16 if ffn_bf16 else F32R
    d_wfc = nc.declare_dram_parameter("wfc", [L, 128, KD, FFL], WDT, isOutput=False)
    d_ffc = nc.declare_dram_parameter("ffc", [L, 2, FFL], F32R, isOutput=False)
    d_wout = nc.declare_dram_parameter("wout", [L, 128, KF, D], WDT, isOutput=False)
    d_whead = nc.declare_dram_parameter("whead", [128, KD, VL], BF16, isOutput=False)
    d_fhead = nc.declare_dram_parameter("fhead", [2, VL], F32R, isOutput=False)
    d_ones_col = nc.declare_dram_parameter("ones_col", [128, 1], F32R, isOutput=False)
    d_ones_colh = nc.declare_dram_parameter("ones_colh", [128, 1], BF16, isOutput=False)
    d_ones_row = nc.declare_dram_parameter("ones_row", [1, 128], F32R, isOutput=False)
    d_gb = nc.declare_dram_parameter("gb", [L, 128, GB_W], F32, isOutput=False)
    d_logits = nc.declare_dram_parameter("logits", [VL, N], F32, isOutput=True)

    ar_in, ar_out = {}, {}
    for l in range(L):
        for s in range(2):
            for b in range(B):
                ar_in[l, s, b] = nc.dram_tensor(
                    f"arin_{l}_{s}_{b}", [128, KD * T], F16
                )
                ar_out[l, s, b] = nc.dram_tensor(
                    f"arout_{l}_{s}_{b}", [128, KD * T], F16, addr_space="Shared"
                )

    AF = mybir.ActivationFunctionType
    ALU = mybir.AluOpType

    with tile.TileContext(nc) as tc, ExitStack() as ctx:
        res = ctx.enter_context(tc.tile_pool(name="res", bufs=1))
        cst = ctx.enter_context(tc.tile_pool(name="cst", bufs=2))
        a1 = ctx.enter_context(tc.tile_pool(name="a1", bufs=1))
        a2 = ctx.enter_context(tc.tile_pool(name="a2", bufs=2))
        a3 = ctx.enter_context(tc.tile_pool(name="a3", bufs=1))
        wgt = ctx.enter_context(tc.tile_pool(name="wgt", bufs=1))
        ps2 = ctx.enter_context(tc.tile_pool(name="ps2", bufs=2, space="PSUM"))
        psb = ctx.enter_context(tc.tile_pool(name="psb", bufs=2, space="PSUM"))
        ps3 = ctx.enter_context(tc.tile_pool(name="ps3", bufs=4, space="PSUM"))

        ones_col = res.tile([128, 1], F32R, tag="ones_col")
        nc.sync.dma_start(out=ones_col[:], in_=d_ones_col[:])
        ones_colh = res.tile([128, 1], BF16, tag="ones_colh")
        nc.sync.dma_start(out=ones_colh[:], in_=d_ones_colh[:])
        ones_row = res.tile([1, 128], F32R, tag="ones_row")
        nc.sync.dma_start(out=ones_row[:], in_=d_ones_row[:])
        ident = res.tile([128, 128], F32, tag="ident")
        make_identity(nc, ident[:])
        eps_col = res.tile([1, 1], F32, tag="eps")
        nc.vector.memset(eps_col[:], EPS)

        # ---- LN stats helpers (fold: no h tiles, stats feed rank-2) ----
        def stats_trees(xb, b):
            """vector partial-sum trees for sum(x) and sum(x^2)."""
            sqb = a2.tile([128, KD, NCH], BF16, tag="sqb", bufs=1)
            nc.scalar.activation(sqb[:], xb[:].bitcast(F32), AF.Square)
            t6x = a2.tile([128, 6, NCH], F32R, tag="t6x", bufs=1)
            nc.vector.tensor_add(
                t6x[:], xb[:, 0:6, :].bitcast(F32), xb[:, 6:12, :].bitcast(F32)
            )
            t3x = a2.tile([128, 3, NCH], F32R, tag="t3x", bufs=1)
            nc.vector.tensor_add(
                t3x[:], t6x[:, 0:3, :].bitcast(F32), t6x[:, 3:6, :].bitcast(F32)
            )
            t2x = a2.tile([128, NCH], F32R, tag="t2x", bufs=1)
            nc.vector.tensor_add(
                t2x[:], t3x[:, 0, :].bitcast(F32), t3x[:, 1, :].bitcast(F32)
            )
            t1x = a2.tile([128, NCH], F32R, tag="t1x", bufs=1)
            nc.vector.tensor_add(
                t1x[:], t2x[:].bitcast(F32), t3x[:, 2, :].bitcast(F32)
            )
            with nc.allow_low_precision("bf16 sq tree"):
                t6q = a2.tile([128, 6, NCH], BF16, tag="t6q", bufs=1)
                nc.vector.tensor_add(t6q[:], sqb[:, 0:6, :], sqb[:, 6:12, :])
                t3q = a2.tile([128, 3, NCH], BF16, tag="t3q", bufs=1)
                nc.vector.tensor_add(t3q[:], t6q[:, 0:3, :], t6q[:, 3:6, :])
                t2q = a2.tile([128, NCH], BF16, tag="t2q", bufs=1)
                nc.vector.tensor_add(t2q[:], t3q[:, 0, :], t3q[:, 1, :])
                t1q = a2.tile([128, NCH], BF16, tag="t1q", bufs=1)
                nc.vector.tensor_add(t1q[:], t2q[:], t3q[:, 2, :])
            return t1x, t1q

        def ln_rows(t1x, t1q, b):
            """tensor reduces + row chain -> m_row, std_row, r_row [1,NCH]."""
            s_p = ps2.tile([1, NCH], F32, tag="row")
            nc.tensor.matmul(s_p[:], ones_col[:], t1x[:], start=True, stop=True)
            q_p = ps2.tile([1, NCH], F32, tag="row")
            nc.tensor.matmul(q_p[:], ones_colh[:], t1q[:], start=True, stop=True)
            m_row = a1.tile([1, NCH], F32R, tag=f"m_row{b}")
            nc.vector.tensor_scalar(m_row[:], s_p[:], 1.0 / D, None, ALU.mult)
            ms = a1.tile([1, NCH], F32, tag=f"ms{b}")
            nc.vector.tensor_mul(ms[:], m_row[:].bitcast(F32), m_row[:].bitcast(F32))
            var = a1.tile([1, NCH], F32, tag=f"var{b}")
            nc.vector.scalar_tensor_tensor(
                var[:], q_p[:], 1.0 / D, ms[:], ALU.mult, ALU.subtract
            )
            std_row = a1.tile([1, NCH], F32R, tag=f"std_row{b}")
            nc.scalar.activation(std_row[:], var[:], AF.Sqrt, bias=eps_col[:])
            r_row = a1.tile([1, NCH], F32, tag=f"r_row{b}")
            nc.vector.reciprocal_approx_fast(r_row[:], std_row[:].bitcast(F32))
            rr_row = a1.tile([1, NCH], F32R, tag=f"rr_row{b}")
            nc.vector.tensor_scalar(rr_row[:], r_row[:], 1.0, None, ALU.mult)
            return m_row, std_row, r_row, rr_row

        def rb_bcast(rr_row, b):
            p_rb = psb.tile([128, NCH], F32, tag="bc")
            nc.tensor.matmul(
                p_rb[:], ones_row[:], rr_row[:], start=True, stop=True
            )
            rb_s = a1.tile([128, NCH], F32, tag=f"rb_s{b}")
            nc.scalar.copy(rb_s[:], p_rb[:])
            return rb_s

        def partial_out(psum, gb_t, boff, mi, stage):
            """stage[:, mi, :] = psum + b/8 in fp16 (alternate engines)."""
            if mi % 2 == 0:
                with nc.allow_low_precision("fp16 allreduce payload"):
                    nc.vector.tensor_scalar(
                        stage[:, mi, :], psum[:, 0:T],
                        gb_t[:, boff + mi:boff + mi + 1], None, ALU.add
                    )
            else:
                nc.scalar.activation(
                    stage[:, mi, :], psum[:, 0:T], AF.Identity,
                    bias=gb_t[:, boff + mi:boff + mi + 1],
                )

        def all_reduce(l, s, b):
            if use_cc:
                nc.gpsimd.collective_compute(
                    "AllReduce", ALU.add,
                    replica_groups=[list(range(NC))],
                    ins=[ar_in[l, s, b][:].opt()],
                    outs=[ar_out[l, s, b][:].opt()],
                )
            else:
                nc.gpsimd.dma_start(out=ar_out[l, s, b][:], in_=ar_in[l, s, b][:])

        def refresh_xt(xb, l, s, b):
            tmp = a3.tile([128, KD, T], F16, tag=f"stage{b}")
            QK = KD // 4
            for q in range(4):
                k0, k1 = q * QK, (q + 1) * QK
                nc.sync.dma_start(
                    out=tmp[:, k0:k1, :], in_=ar_out[l, s, b][:, k0 * T:k1 * T]
                )
                nc.vector.tensor_add(
                    xb[:, k0:k1, 0:T], xb[:, k0:k1, 0:T].bitcast(F32),
                    tmp[:, k0:k1, :]
                )

        def xh_copy(xb, b):
            xh = a1.tile([128, KD, NCH], BF16, tag=f"xh{b}")
            HK = KD // 2
            nc.scalar.copy(xh[:, 0:HK, :], xb[:, 0:HK, :].bitcast(F32))
            nc.scalar.copy(xh[:, HK:KD, :], xb[:, HK:KD, :].bitcast(F32))
            return xh

        warm_in = nc.dram_tensor("warm_in", [128, 128], F16)
        warm_out = nc.dram_tensor("warm_out", [128, 128], F16, addr_space="Shared")


        def load_attn_weights(l):
            gb_t = cst.tile([128, GB_W], F32, tag="gb")
            nc.sync.dma_start(out=gb_t[:], in_=d_gb[l])
            wa = wgt.tile([128, KD, 256], F32R, tag="wa")
            nc.sync.dma_start(out=wa[:], in_=d_wattn[l])
            fat1 = cst.tile([1, 256], F32R, tag="fat1", bufs=1)
            nc.sync.dma_start(out=fat1[:], in_=d_fattn[l, 0:1, :])
            fat2 = cst.tile([1, 256], F32R, tag="fat2", bufs=1)
            nc.sync.dma_start(out=fat2[:], in_=d_fattn[l, 1:2, :])
            wp = wgt.tile([DH, HPC, D], BF16, tag="wp")
            nc.sync.dma_start(out=wp[:], in_=d_wproj[l])
            return gb_t, wa, fat1, fat2, wp

        def load_ffn_weights(l):
            wfc_t = wgt.tile([128, KD, FFL], WDT, tag="wfc")
            nc.sync.dma_start(out=wfc_t[:], in_=d_wfc[l])
            ffc1 = cst.tile([1, FFL], F32R, tag="ffc1", bufs=1)
            nc.sync.dma_start(out=ffc1[:], in_=d_ffc[l, 0:1, :])
            ffc2 = cst.tile([1, FFL], F32R, tag="ffc2", bufs=1)
            nc.sync.dma_start(out=ffc2[:], in_=d_ffc[l, 1:2, :])
            wout_t = wgt.tile([128, KF, D], WDT, tag="wout")
            nc.sync.dma_start(out=wout_t[:], in_=d_wout[l])
            return wfc_t, ffc1, ffc2, wout_t

        warm_sb = res.tile([128, 128], F16, tag="warm")
        nc.vector.memset(warm_sb[:], 0.0)
        nc.sync.dma_start(out=warm_in[:], in_=warm_sb[:])

        for _rep in range(reps):
            if use_cc:
                nc.gpsimd.collective_compute(
                    "AllReduce", ALU.add,
                    replica_groups=[list(range(NC))],
                    ins=[warm_in[:].opt()],
                    outs=[warm_out[:].opt()],
                )
            xtb = []
            for b in range(B):
                t = res.tile([128, KD, NCH], F32R, name=f"xt{b}", tag=f"xt{b}")
                nc.sync.dma_start(
                    out=t[:], in_=d_x0t[:, :, b * NCH:(b + 1) * NCH]
                )
                xtb.append(t)

            aw = load_attn_weights(0)
            fw = None
            aw_next = None
            for l in range(L):
                gb_t, wa, fat1, fat2, wp = aw

                # ---------- attention sublayer (per batch, pipelined) ------
                for b in range(B):
                    xb = xtb[b]
                    if l > 0:
                        refresh_xt(xb, l - 1, 1, b)
                    if b == 1:
                        fw = load_ffn_weights(l)
                    t1x, t1q = stats_trees(xb, b)

                    # raw Wg^T x groups, stats reduce interleaved
                    pw = []
                    for mi, (ms_, msz) in enumerate(MT):
                        p = ps3.tile([128, 256], F32, tag="mm")
                        for k in range(KD):
                            nc.tensor.matmul(
                                p[:msz, :], xb[:, k, ms_:ms_ + msz], wa[:, k, :],
                                start=(k == 0), stop=False,
                            )
                        pw.append(p)
                        if mi == 1:
                            m_row, std_row, r_row, rr_row = ln_rows(t1x, t1q, b)
                    # rank-1 fixups close each group; r as column per tile
                    rcol = a1.tile([128, 3], F32, tag=f"rcol{b}")
                    whsb = []
                    for mi, (ms_, msz) in enumerate(MT):
                        nc.tensor.matmul(
                            pw[mi][:msz, :],
                            m_row[:, ms_:ms_ + msz], fat1[:],
                            start=False, stop=False,
                        )
                        nc.tensor.matmul(
                            pw[mi][:msz, :],
                            std_row[:, ms_:ms_ + msz], fat2[:],
                            start=False, stop=True,
                        )
                        pt = ps2.tile([128, 1], F32, tag="row")
                        nc.tensor.transpose(
                            pt[:msz, :], r_row[:, ms_:ms_ + msz], ident[:1, :1]
                        )
                        nc.scalar.copy(rcol[:msz, mi:mi + 1], pt[:msz, :])
                        w = a1.tile([128, 198], F32R, tag=f"whsb{mi}_{b}")
                        nc.scalar.activation(
                            w[:msz, 0:196], pw[mi][:msz, 0:196], AF.Identity,
                            scale=rcol[:msz, mi:mi + 1],
                        )
                        nc.vector.tensor_scalar(
                            w[:msz, 196:198], w[:msz, 192:194].bitcast(F32),
                            0.2, None, ALU.mult
                        )
                        whsb.append(w)

                    erow = [
                        a1.tile([1, NCH], F32R, name=f"er{j}_{b}", tag=f"er{j}_{b}")
                        for j in range(HPC)
                    ]
                    for mi, (ms_, msz) in enumerate(MT):
                        for j in range(HPC):
                            pt = ps2.tile([1, 128], F32, tag="row")
                            nc.tensor.transpose(
                                pt[:, :msz],
                                whsb[mi][:msz, 194 + j:195 + j].bitcast(F32),
                                ident[:msz, :msz],
                            )
                            nc.scalar.copy(erow[j][:, ms_:ms_ + msz], pt[:, :msz])

                    aggt = []
                    for j in range(HPC):
                        p_er = psb.tile([128, NCH], F32, tag="bc")
                        nc.tensor.matmul(
                            p_er[:], ones_row[:], erow[j][:], start=True, stop=True
                        )
                        e_tiles = []
                        for mi in range(3):
                            rsz = MT_REAL[mi]
                            e1 = a2.tile([128, NCH], F32R, tag=f"e{mi}_{b}", bufs=1)
                            nc.scalar.activation(
                                e1[:rsz, :], p_er[:rsz, :], AF.Exp,
                                bias=whsb[mi][:rsz, 192 + j:193 + j].bitcast(F32),
                            )
                            e2 = a1.tile([128, NCH], F32, tag="e2")
                            nc.scalar.activation(
                                e2[:rsz, :], p_er[:rsz, :], AF.Exp, scale=0.2,
                                bias=whsb[mi][:rsz, 196 + j:197 + j].bitcast(F32),
                            )
                            nc.vector.tensor_max(
                                e1[:rsz, :], e1[:rsz, :].bitcast(F32), e2[:rsz, :]
                            )
                            e_tiles.append(e1)
                        p_s = ps2.tile([1, NCH], F32, tag="row")
                        for mi in range(3):
                            rsz = MT_REAL[mi]
                            nc.tensor.matmul(
                                p_s[:], ones_col[:rsz, :], e_tiles[mi][:rsz, :],
                                start=(mi == 0), stop=(mi == 2),
                            )
                        sr = a1.tile([1, NCH], F32, tag=f"sr{b}")
                        nc.vector.reciprocal_approx_fast(sr[:], p_s[:])
                        srr = a1.tile([1, NCH], F32R, tag=f"srr{b}")
                        nc.vector.tensor_scalar(srr[:], sr[:], 1.0, None, ALU.mult)
                        p_rb2 = psb.tile([DH, NCH], F32, tag="bc")
                        nc.tensor.matmul(
                            p_rb2[:], ones_row[:, :DH], srr[:],
                            start=True, stop=True,
                        )
                        rb_sb = a1.tile([DH, NCH], F32, tag=f"rb_sb{b}")
                        nc.scalar.copy(rb_sb[:], p_rb2[:])
                        p_agg = ps3.tile([DH, NCH], F32, tag="mm")
                        for mi in range(3):
                            rsz = MT_REAL[mi]
                            nc.tensor.matmul(
                                p_agg[:],
                                whsb[mi][:rsz, j * DH:(j + 1) * DH],
                                e_tiles[mi][:rsz, :],
                                start=(mi == 0), stop=(mi == 2),
                            )
                        at = a1.tile([DH, NCH], BF16, tag=f"aggt{j}_{b}")
                        with nc.allow_low_precision("bf16 agg"):
                            nc.vector.tensor_mul(at[:], p_agg[:], rb_sb[:])
                        aggt.append(at)

                    stage = a3.tile([128, KD, T], F16, tag=f"stage{b}")
                    for mi in range(KD):
                        p = ps3.tile([128, NCH], F32, tag="mm")
                        for j in range(HPC):
                            nc.tensor.matmul(
                                p[:], wp[:, j, mi * 128:(mi + 1) * 128], aggt[j][:],
                                start=(j == 0), stop=(j == HPC - 1),
                            )
                        partial_out(p, gb_t, GB_BPROJ, mi, stage)
                    nc.sync.dma_start(out=ar_in[l, 0, b][:], in_=stage[:])
                    all_reduce(l, 0, b)

                # ---------- FFN sublayer (per batch, pipelined) ------------
                wfc_t, ffc1, ffc2, wout_t = fw

                for b in range(B):
                    xb = xtb[b]
                    refresh_xt(xb, l, 0, b)
                    if b == 1 and l < L - 1:
                        aw_next = load_attn_weights(l + 1)
                    if b == 1 and l == L - 1:
                        fhd1 = cst.tile([1, VL], F32R, tag="fhd1", bufs=1)
                        nc.sync.dma_start(out=fhd1[:], in_=d_fhead[0:1, :])
                        fhd2 = cst.tile([1, VL], F32R, tag="fhd2", bufs=1)
                        nc.sync.dma_start(out=fhd2[:], in_=d_fhead[1:2, :])
                        VLQ = VL // 4
                        wh_pre = wgt.tile([128, KD, VLQ], BF16, tag="whd0")
                        nc.sync.dma_start(
                            out=wh_pre[:], in_=d_whead[:, :, 0:VLQ]
                        )
                    xh = xh_copy(xb, b)
                    t1x, t1q = stats_trees(xb, b)

                    g_tiles = [None] * KF
                    open_p = {}

                    def fc_open(mi):
                        p = ps3.tile([128, NCH], F32, tag="mm")
                        for k in range(KD):
                            nc.tensor.matmul(
                                p[:], wfc_t[:, k, mi * 128:(mi + 1) * 128],
                                xh[:, k, :],
                                start=(k == 0), stop=False,
                            )
                        open_p[mi] = p

                    def fc_close(mi, rb_s):
                        p = open_p.pop(mi)
                        nc.tensor.matmul(
                            p[:], ffc1[:, mi * 128:(mi + 1) * 128],
                            m_row[:],
                            start=False, stop=False,
                        )
                        nc.tensor.matmul(
                            p[:], ffc2[:, mi * 128:(mi + 1) * 128],
                            std_row[:],
                            start=False, stop=True,
                        )
                        gf_ = a2.tile([128, NCH], F32, tag="gf")
                        nc.vector.tensor_mul(gf_[:], p[:], rb_s[:])
                        g = a2.tile([128, NCH], WDT, tag=f"g{mi}_{b}", bufs=1)
                        nc.scalar.activation(
                            g[:], gf_[:], AF.Gelu,
                            bias=gb_t[:, GB_BFC + mi:GB_BFC + mi + 1],
                        )
                        g_tiles[mi] = g

                    fc_open(0)
                    fc_open(1)
                    m_row, std_row, r_row, rr_row = ln_rows(t1x, t1q, b)
                    fc_open(2)
                    fc_open(3)
                    rb_s = rb_bcast(rr_row, b)
                    fc_close(0, rb_s)
                    fc_open(4)
                    fc_close(1, rb_s)
                    fc_open(5)
                    fc_close(2, rb_s)
                    fc_close(3, rb_s)
                    fc_close(4, rb_s)
                    fc_close(5, rb_s)

                    stage = a3.tile([128, KD, T], F16, tag=f"stage{b}")
                    for mi in range(KD):
                        p = ps3.tile([128, NCH], F32, tag="mm")
                        for k in range(KF):
                            nc.tensor.matmul(
                                p[:], wout_t[:, k, mi * 128:(mi + 1) * 128],
                                g_tiles[k][:],
                                start=(k == 0), stop=(k == KF - 1),
                            )
                        partial_out(p, gb_t, GB_BOUT, mi, stage)
                    nc.sync.dma_start(out=ar_in[l, 1, b][:], in_=stage[:])
                    all_reduce(l, 1, b)
                if l < L - 1:
                    aw = aw_next

            # ---------- final LN + vocab-sharded head ----------
            KVQ = KV // 4

            def head_mms(wh_t, xh, m_row, std_row, rb_s, b, quarter):
                for mi in range(KVQ):
                    gmi = quarter * KVQ + mi
                    p = ps3.tile([128, NCH], F32, tag="mm")
                    for k in range(KD):
                        nc.tensor.matmul(
                            p[:], wh_t[:, k, mi * 128:(mi + 1) * 128],
                            xh[:, k, :],
                            start=(k == 0), stop=False,
                        )
                    nc.tensor.matmul(
                        p[:], fhd1[:, gmi * 128:(gmi + 1) * 128],
                        m_row[:],
                        start=False, stop=False,
                    )
                    nc.tensor.matmul(
                        p[:], fhd2[:, gmi * 128:(gmi + 1) * 128],
                        std_row[:],
                        start=False, stop=True,
                    )
                    lg = a2.tile([128, NCH], F32, tag=f"part1{b}", bufs=1)
                    nc.vector.tensor_mul(lg[:], p[:], rb_s[:])
                    nc.sync.dma_start(
                        out=d_logits[gmi * 128:(gmi + 1) * 128, b * T:(b + 1) * T],
                        in_=lg[:, 0:T],
                    )

            qi = 0
            for b in range(B):
                refresh_xt(xtb[b], L - 1, 1, b)
                xh = xh_copy(xtb[b], b)
                t1x, t1q = stats_trees(xtb[b], b)
                m_row, std_row, r_row, rr_row = ln_rows(t1x, t1q, b)
                rb_s = rb_bcast(rr_row, b)
                for quarter in range(4):
                    if qi == 0:
                        wh_t = wh_pre
                    else:
                        wh_t = wgt.tile(
                            [128, KD, VLQ], BF16, tag=f"whd{qi % 2}"
                        )
                        nc.sync.dma_start(
                            out=wh_t[:],
                            in_=d_whead[:, :, quarter * VLQ:(quarter + 1) * VLQ],
                        )
                    head_mms(wh_t, xh, m_row, std_row, rb_s, b, quarter)
                    qi += 1

    nc.compile()
    return nc


def _get_nc(reps=1, use_cc=True, ffn_bf16=True):
    key = f"nc{reps}_{use_cc}_{ffn_bf16}"
    if key not in _CACHE:
        _CACHE[key] = _build_nc(reps, use_cc, ffn_bf16)
    return _CACHE[key]


# --------------------------------------------------------------------------
# numpy fallback (exact reference semantics for arbitrary edges)
# --------------------------------------------------------------------------

def _numpy_forward(inp):
    from scipy.special import erf

    def ln(x, g, b):
        m = x.mean(-1, keepdims=True)
        v = ((x - m) ** 2).mean(-1, keepdims=True)
        return (x - m) / np.sqrt(v + EPS) * g + b

    f32 = np.float32
    objs_e = np.asarray(inp["obj_emb_w"])[np.asarray(inp["objs"])]
    pe = np.asarray(inp["poss_emb_w"])[np.asarray(inp["poss"])]
    nfeat = np.concatenate([objs_e, pe[:, :NOBJ], pe[:, NOBJ:]], axis=-1)
    z = np.asarray(inp["tok_emb"])[np.asarray(inp["z_indices"])]
    x = np.concatenate([nfeat, z], axis=1) + np.asarray(inp["pos_emb"])[:, :T]
    x = x.reshape(N, D).astype(f32)
    src = np.asarray(inp["src"]).astype(np.int64)
    dst = np.asarray(inp["dst"]).astype(np.int64)
    for l in range(L):
        h = ln(x, inp["ln1_g"][l], inp["ln1_b"][l])
        Wh = (h @ np.asarray(inp["W_attn"][l])).reshape(N, H, DH)
        el = np.einsum("nhd,hd->nh", Wh, np.asarray(inp["a_l"][l]))
        er = np.einsum("nhd,hd->nh", Wh, np.asarray(inp["a_r"][l]))
        e = el[src] + er[dst]
        e = np.where(e >= 0, e, 0.2 * e)
        m = np.full((N, H), -np.inf, f32)
        np.maximum.at(m, dst, e)
        m[~np.isfinite(m)] = 0.0
        ex = np.exp(e - m[dst])
        s = np.zeros((N, H), f32)
        np.add.at(s, dst, ex)
        alpha = ex / s[dst]
        agg = np.zeros((N, H, DH), f32)
        np.add.at(agg, dst, alpha[:, :, None] * Wh[src])
        x = x + agg.reshape(N, D) @ np.asarray(inp["W_proj"][l]) \
            + np.asarray(inp["b_proj"][l])
        h2 = ln(x, inp["ln2_g"][l], inp["ln2_b"][l])
        ff = h2 @ np.asarray(inp["W_fc"][l]) + np.asarray(inp["b_fc"][l])
        ff = ff * 0.5 * (1.0 + erf(ff / np.sqrt(2.0)))
        x = x + ff @ np.asarray(inp["W_out"][l]) + np.asarray(inp["b_out"][l])
    x = ln(x, inp["lnf_g"], inp["lnf_b"])
    return (x @ np.asarray(inp["head_w"])).reshape(B, T, V).astype(f32)


# --------------------------------------------------------------------------
# public entry
# --------------------------------------------------------------------------

def _edges_are_block_diag(inp):
    src, dst = _block_diag_edges_np()
    s = np.asarray(inp["src"])
    d = np.asarray(inp["dst"])
    return (
        s.shape == src.shape
        and np.array_equal(s.astype(np.int64), src)
        and np.array_equal(d.astype(np.int64), dst)
    )


def _assemble(results):
    full = np.concatenate([results[c]["logits"] for c in range(NC)], axis=0)
    return np.ascontiguousarray(full.T).reshape(B, T, V)


def kernel(**inputs):
    if not _edges_are_block_diag(inputs):
        return _numpy_forward(inputs)
    from concourse import bass2jax

    in_maps = _host_inputs(inputs)
    results = bass2jax.run_bass_via_pjrt(_get_nc(), in_maps, n_cores=NC)
    return _assemble(results)


# --------------------------------------------------------------------------
# benchmarking (repeated execution, device-resident inputs)
# --------------------------------------------------------------------------

def _make_runner(nc):
    """Persistent jitted shard_map callable for nc (multi-core), mirroring
    bass2jax.run_bass_via_pjrt but reusable across calls."""
    import jax
    from jax.sharding import Mesh, PartitionSpec
    from jax.experimental.shard_map import shard_map
    from concourse import bass2jax, mybir as _mybir

    bass2jax.install_neuronx_cc_hook()
    partition_name = nc.partition_id_tensor.name if nc.partition_id_tensor else None
    in_names, out_names, out_avals, zero_outs = [], [], [], []
    for alloc in nc.m.functions[0].allocations:
        if not isinstance(alloc, _mybir.MemoryLocationSet):
            continue
        name = alloc.memorylocations[0].name
        if alloc.kind == "ExternalInput":
            if name != partition_name:
                in_names.append(name)
        elif alloc.kind == "ExternalOutput":
            shape = tuple(alloc.tensor_shape)
            dtype = _mybir.dt.np(alloc.dtype)
            out_names.append(name)
            out_avals.append(jax.core.ShapedArray(shape, dtype))
            zero_outs.append(np.zeros(shape, dtype))
    n_params = len(in_names)
    all_in_names = list(in_names) + list(out_names)
    if partition_name is not None:
        all_in_names.append(partition_name)

    def _body(*args):
        operands = list(args)
        if partition_name is not None:
            operands.append(bass2jax.partition_id_tensor())
        return tuple(
            bass2jax._bass_exec_p.bind(
                *operands,
                out_avals=tuple(out_avals),
                in_names=tuple(all_in_names),
                out_names=tuple(out_names),
                lowering_input_output_aliases=(),
                sim_require_finite=True,
                sim_require_nnan=True,
                nc=nc,
            )
        )

    devices = jax.devices()[:NC]
    mesh = Mesh(np.asarray(devices), ("core",))
    n_outs = len(out_names)
    in_specs = (PartitionSpec("core"),) * (n_params + n_outs)
    out_specs = (PartitionSpec("core"),) * n_outs
    donate = tuple(range(n_params, n_params + n_outs))
    fn = jax.jit(
        shard_map(_body, mesh=mesh, in_specs=in_specs, out_specs=out_specs,
                  check_rep=False),
        donate_argnums=donate, keep_unused=True,
    )
    return fn, in_names, out_names, zero_outs, mesh


def _timed_run(nc, in_maps, iters):
    """Median wall time (s) per execution with device-resident inputs."""
    import jax

    from jax.sharding import NamedSharding, PartitionSpec

    fn, in_names, out_names, zero_outs, mesh = _make_runner(nc)
    shard = NamedSharding(mesh, PartitionSpec("core"))
    concat_in = [
        np.concatenate([np.asarray(m[name]) for m in in_maps], axis=0)
        for name in in_names
    ]
    dev_in = [jax.device_put(a, shard) for a in concat_in]
    jax.block_until_ready(dev_in)

    def zeros():
        zs = [
            jax.device_put(
                np.zeros((NC * z.shape[0], *z.shape[1:]), z.dtype), shard
            )
            for z in zero_outs
        ]
        jax.block_until_ready(zs)
        return zs

    outs = fn(*dev_in, *zeros())  # warm-up/compile
    jax.block_until_ready(outs)
    times = []
    for _ in range(iters):
        zs = zeros()
        t0 = time.perf_counter()
        outs = fn(*dev_in, *zs)
        jax.block_until_ready(outs)
        times.append(time.perf_counter() - t0)
    return float(np.min(times)), outs, out_names


def bench(inputs, iters=16):
    """HW ns per network pass via reps-differential (cancels dispatch cost)."""
    in_maps = _host_inputs(inputs)
    t1, _, _ = _timed_run(_get_nc(1), in_maps, iters)
    t9, _, _ = _timed_run(_get_nc(9), in_maps, iters)
    print(f"  wall/iter reps1: {t1 * 1e6:.0f} us,  reps9: {t9 * 1e6:.0f} us")
    return max(t9 - t1, 0.0) / 8 * 1e9


# revision 40
# speedup vs baseline: 1.0360x; 1.0360x over previous
"""Trainium2 Bass kernel for nn_GAT_42786464203341.

8-way tensor parallel (Megatron-style) over one trn2 chip:
  - The GAT edges are block-diagonal fully-connected per sample, so message
    passing is dense per-sample attention with scores leaky(el[i] + er[j]),
    softmaxed over source i.
  - Activations are feature-major (x^T: [D, nodes]); contraction on
    partitions feeds the PE array directly.
  - LayerNorm is folded into the following matmul: with Wg = diag(g)W,
    y = r*(Wg^T x) + m*f1 + std*f2 scaled by r, where f1 = -colsum(Wg),
    f2 = W^T b_ln.  The rank-2 term is accumulated into the same PSUM
    group (lhsT/rhs = [m_row; std_row] x [f1; f2]); the *r scale happens
    at the PSUM->SBUF copy.  LN stats come from vector-engine partial-sum
    trees + one ones-vector matmul each for sum and sum-of-squares.
  - Attention is head-parallel (2 heads/core); W_proj row-sharded ->
    partial [D, nodes] -> AllReduce.  FFN column/row sharded -> AllReduce.
    Head is vocab-sharded; host concatenates the 8 logits slices.
  - AllReduces are split per batch (12 total) and software-pipelined:
    AR(batch 0) overlaps compute of batch 1 and vice versa.
  - Weight / staging / refresh DMAs are batched into single descriptors
    (host pre-permutes weights into [128, ktiles, cols] layout).
"""

import time
from contextlib import ExitStack

import ml_dtypes
import numpy as np

import concourse.bass as bass
import concourse.tile as tile
from concourse import bacc, mybir
from concourse.masks import make_identity

F32 = mybir.dt.float32
F32R = mybir.dt.float32r
F16 = mybir.dt.float16
BF16 = mybir.dt.bfloat16

B, T, NOBJ = 2, 265, 9
D, H, DH = 1536, 16, 96
V, PV, L, FF = 8192, 512, 3, 6144
N = B * T          # 530
NC = 8             # cores
HPC = H // NC      # heads per core
FFL = FF // NC     # 768
VL = V // NC       # 1024
NCH = T + 1        # 266 (col 265 of each chunk is zero padding)
NP = B * NCH       # 532
KD = D // 128      # 12
KF = FFL // 128    # 6
KV = VL // 128     # 8
MT = [(0, 128), (128, 128), (256, 10)]   # node tiles per batch (start, size)
MT_REAL = [128, 128, 9]                  # non-pad rows per node tile
EPS = 1e-5

# gb blob column offsets: bout8, bproj8 (KD each), bfc (KF)
GB_BOUT, GB_BPROJ, GB_BFC = 0, KD, 2 * KD
GB_W = 2 * KD + KF

_CACHE = {}


# --------------------------------------------------------------------------
# host-side input prep
# --------------------------------------------------------------------------

def _block_diag_edges_np():
    base = np.arange(T)
    src = np.concatenate([g * T + np.repeat(base, T) for g in range(B)])
    dst = np.concatenate([g * T + np.tile(base, T) for g in range(B)])
    return src.astype(np.int64), dst.astype(np.int64)


def _perm_k(w, ktiles):
    """[.., K*128, cols] -> [.., 128, ktiles, cols] (partition-major)."""
    s = w.shape
    return np.ascontiguousarray(
        w.reshape(*s[:-2], ktiles, 128, s[-1]).swapaxes(-3, -2)
    )


def _host_inputs(inp, ffn_bf16=True):
    f32 = np.float32
    bf16 = ml_dtypes.bfloat16
    objs_e = np.asarray(inp["obj_emb_w"])[np.asarray(inp["objs"])]
    pe = np.asarray(inp["poss_emb_w"])[np.asarray(inp["poss"])]
    nfeat = np.concatenate([objs_e, pe[:, :NOBJ], pe[:, NOBJ:]], axis=-1)
    z = np.asarray(inp["tok_emb"])[np.asarray(inp["z_indices"])]
    x0 = np.concatenate([nfeat, z], axis=1) + np.asarray(inp["pos_emb"])[:, :T]
    x0 = x0.reshape(N, D).astype(f32)

    x0t = np.zeros((D, NP), f32)
    for b in range(B):
        x0t[:, b * NCH:b * NCH + T] = x0[b * T:(b + 1) * T].T
    x0tp = _perm_k(x0t, KD)                       # [128, KD, NP]

    W_attn = np.asarray(inp["W_attn"], f32)
    a_l = np.asarray(inp["a_l"], f32)
    a_r = np.asarray(inp["a_r"], f32)
    W_proj = np.asarray(inp["W_proj"], f32)
    W_fc = np.asarray(inp["W_fc"], f32)
    W_out = np.asarray(inp["W_out"], f32)
    head_w = np.asarray(inp["head_w"], f32)
    g1 = np.asarray(inp["ln1_g"], f32)            # [L, D]
    b1 = np.asarray(inp["ln1_b"], f32)
    g2 = np.asarray(inp["ln2_g"], f32)
    b2 = np.asarray(inp["ln2_b"], f32)
    gf = np.asarray(inp["lnf_g"], f32)            # [D]
    bf = np.asarray(inp["lnf_b"], f32)

    def cols(vec, k_tiles):  # [3, D'] -> [3, 128, k_tiles]
        v = np.asarray(vec, f32)
        return np.transpose(v.reshape(3, k_tiles, 128), (0, 2, 1)).copy()

    bfc_all = cols(inp["b_fc"], KF * NC)
    bout8 = cols(np.asarray(inp["b_out"], f32) / NC, KD)
    bproj8 = cols(np.asarray(inp["b_proj"], f32) / NC, KD)

    # LN2-folded FC weights and fixups (full, sliced per core below)
    wfc_g_full = W_fc * g2[:, :, None]            # [L, D, FF]
    f1_fc_full = -wfc_g_full.sum(axis=1)          # [L, FF]
    f2_fc_full = np.einsum("ld,ldo->lo", b2, W_fc)
    # LNF-folded head weights and fixups
    whead_g = head_w * gf[:, None]                # [D, V]
    f1_hd_full = -whead_g.sum(axis=0)             # [V]
    f2_hd_full = bf @ head_w                      # [V]

    wdt = bf16 if ffn_bf16 else f32
    maps = []
    for c in range(NC):
        h0 = c * HPC
        wattn = np.zeros((L, D, 256), f32)
        for j in range(HPC):
            hg = h0 + j
            blk = W_attn[:, :, hg * DH:(hg + 1) * DH]         # [3, D, DH]
            wattn[:, :, j * DH:(j + 1) * DH] = blk
            # el/er are linear in h: fold (W_attn-block @ a) into one column
            wattn[:, :, 192 + j] = np.matmul(blk, a_l[:, hg, :, None])[..., 0]
            wattn[:, :, 194 + j] = np.matmul(blk, a_r[:, hg, :, None])[..., 0]
        # LN1 fold for the attention matmul
        f2_at = np.einsum("ld,ldo->lo", b1, wattn)            # [L, 256]
        wattn_g = wattn * g1[:, :, None]
        f1_at = -wattn_g.sum(axis=1)                          # [L, 256]
        f_attn = np.stack([f1_at, f2_at], axis=1)             # [L, 2, 256]

        wproj = np.stack(
            [W_proj[:, (h0 + j) * DH:(h0 + j + 1) * DH, :] for j in range(HPC)],
            axis=2,
        )                                          # [L, DH, HPC, D]
        gb = np.concatenate(
            [bout8, bproj8, bfc_all[:, :, c * KF:(c + 1) * KF]],
            axis=2,
        ).copy()                                   # [L, 128, GB_W]
        fsl = slice(c * FFL, (c + 1) * FFL)
        vsl = slice(c * VL, (c + 1) * VL)
        maps.append({
            "x0t": x0tp,
            "wattn": _perm_k(wattn_g, KD).astype(f32),   # f32r on device
            "fattn": np.ascontiguousarray(f_attn),
            "wproj": np.ascontiguousarray(wproj).astype(bf16),
            "wfc": _perm_k(
                np.ascontiguousarray(wfc_g_full[:, :, fsl]), KD
            ).astype(wdt),                         # [L, 128, KD, FFL]
            "ffc": np.ascontiguousarray(
                np.stack([f1_fc_full[:, fsl], f2_fc_full[:, fsl]], axis=1)
            ),                                     # [L, 2, FFL]
            "wout": _perm_k(
                np.ascontiguousarray(W_out[:, fsl, :]), KF
            ).astype(wdt),                         # [L, 128, KF, D]
            "whead": _perm_k(
                np.ascontiguousarray(whead_g[:, vsl]), KD
            ).astype(bf16),                        # [128, KD, VL]
            "fhead": np.ascontiguousarray(
                np.stack([f1_hd_full[vsl], f2_hd_full[vsl]], axis=0)
            ),                                     # [2, VL]
            "ones_col": np.ones((128, 1), f32),
            "ones_colh": np.ones((128, 1), bf16),
            "ones_row": np.ones((1, 128), f32),
            "gb": gb,
        })
    return maps


# --------------------------------------------------------------------------
# device program
# --------------------------------------------------------------------------

def _build_nc(reps=1, use_cc=True, ffn_bf16=True):
    nc = bacc.Bacc("TRN2", target_bir_lowering=False, debug=False, num_devices=NC)

    d_x0t = nc.declare_dram_parameter("x0t", [128, KD, NP], F32R, isOutput=False)
    d_wattn = nc.declare_dram_parameter("wattn", [L, 128, KD, 256], F32R, isOutput=False)
    d_fattn = nc.declare_dram_parameter("fattn", [L, 2, 256], F32R, isOutput=False)
    d_wproj = nc.declare_dram_parameter("wproj", [L, DH, HPC, D], BF16, isOutput=False)
    WDT = BF16 if ffn_bf16 else F32R
    d_wfc = nc.declare_dram_parameter("wfc", [L, 128, KD, FFL], WDT, isOutput=False)
    d_ffc = nc.declare_dram_parameter("ffc", [L, 2, FFL], F32R, isOutput=False)
    d_wout = nc.declare_dram_parameter("wout", [L, 128, KF, D], WDT, isOutput=False)
    d_whead = nc.declare_dram_parameter("whead", [128, KD, VL], BF16, isOutput=False)
    d_fhead = nc.declare_dram_parameter("fhead", [2, VL], F32R, isOutput=False)
    d_ones_col = nc.declare_dram_parameter("ones_col", [128, 1], F32R, isOutput=False)
    d_ones_colh = nc.declare_dram_parameter("ones_colh", [128, 1], BF16, isOutput=False)
    d_ones_row = nc.declare_dram_parameter("ones_row", [1, 128], F32R, isOutput=False)
    d_gb = nc.declare_dram_parameter("gb", [L, 128, GB_W], F32, isOutput=False)
    d_logits = nc.declare_dram_parameter("logits", [VL, N], F32, isOutput=True)

    ar_in, ar_out = {}, {}
    for l in range(L):
        for s in range(2):
            for b in range(B):
                ar_in[l, s, b] = nc.dram_tensor(
                    f"arin_{l}_{s}_{b}", [128, KD * T], F16
                )
                ar_out[l, s, b] = nc.dram_tensor(
                    f"arout_{l}_{s}_{b}", [128, KD * T], F16, addr_space="Shared"
                )

    AF = mybir.ActivationFunctionType
    ALU = mybir.AluOpType

    with tile.TileContext(nc) as tc, ExitStack() as ctx:
        res = ctx.enter_context(tc.tile_pool(name="res", bufs=1))
        cst = ctx.enter_context(tc.tile_pool(name="cst", bufs=2))
        a1 = ctx.enter_context(tc.tile_pool(name="a1", bufs=1))
        a2 = ctx.enter_context(tc.tile_pool(name="a2", bufs=2))
        a3 = ctx.enter_context(tc.tile_pool(name="a3", bufs=1))
        wgt = ctx.enter_context(tc.tile_pool(name="wgt", bufs=1))
        ps2 = ctx.enter_context(tc.tile_pool(name="ps2", bufs=2, space="PSUM"))
        psb = ctx.enter_context(tc.tile_pool(name="psb", bufs=2, space="PSUM"))
        ps3 = ctx.enter_context(tc.tile_pool(name="ps3", bufs=4, space="PSUM"))

        ones_col = res.tile([128, 1], F32R, tag="ones_col")
        nc.sync.dma_start(out=ones_col[:], in_=d_ones_col[:])
        ones_colh = res.tile([128, 1], BF16, tag="ones_colh")
        nc.sync.dma_start(out=ones_colh[:], in_=d_ones_colh[:])
        ones_row = res.tile([1, 128], F32R, tag="ones_row")
        nc.sync.dma_start(out=ones_row[:], in_=d_ones_row[:])
        ident = res.tile([128, 128], F32, tag="ident")
        make_identity(nc, ident[:])
        eps_col = res.tile([1, 1], F32, tag="eps")
        nc.vector.memset(eps_col[:], EPS)

        # ---- LN stats helpers (fold: no h tiles, stats feed rank-2) ----
        def stats_trees(xb, b):
            """vector partial-sum trees for sum(x) and sum(x^2)."""
            sqb = a2.tile([128, KD, NCH], BF16, tag="sqb", bufs=1)
            nc.scalar.activation(sqb[:], xb[:].bitcast(F32), AF.Square)
            t6x = a2.tile([128, 6, NCH], F32R, tag="t6x", bufs=1)
            nc.vector.tensor_add(
                t6x[:], xb[:, 0:6, :].bitcast(F32), xb[:, 6:12, :].bitcast(F32)
            )
            t3x = a2.tile([128, 3, NCH], F32R, tag="t3x", bufs=1)
            nc.vector.tensor_add(
                t3x[:], t6x[:, 0:3, :].bitcast(F32), t6x[:, 3:6, :].bitcast(F32)
            )
            t2x = a2.tile([128, NCH], F32R, tag="t2x", bufs=1)
            nc.vector.tensor_add(
                t2x[:], t3x[:, 0, :].bitcast(F32), t3x[:, 1, :].bitcast(F32)
            )
            t1x = a2.tile([128, NCH], F32R, tag="t1x", bufs=1)
            nc.vector.tensor_add(
                t1x[:], t2x[:].bitcast(F32), t3x[:, 2, :].bitcast(F32)
            )
            with nc.allow_low_precision("bf16 sq tree"):
                t6q = a2.tile([128, 6, NCH], BF16, tag="t6q", bufs=1)
                nc.vector.tensor_add(t6q[:], sqb[:, 0:6, :], sqb[:, 6:12, :])
                t3q = a2.tile([128, 3, NCH], BF16, tag="t3q", bufs=1)
                nc.vector.tensor_add(t3q[:], t6q[:, 0:3, :], t6q[:, 3:6, :])
                t2q = a2.tile([128, NCH], BF16, tag="t2q", bufs=1)
                nc.vector.tensor_add(t2q[:], t3q[:, 0, :], t3q[:, 1, :])
                t1q = a2.tile([128, NCH], BF16, tag="t1q", bufs=1)
                nc.vector.tensor_add(t1q[:], t2q[:], t3q[:, 2, :])
            return t1x, t1q

        def ln_rows(t1x, t1q, b):
            """tensor reduces + row chain -> m_row, std_row, r_row [1,NCH]."""
            s_p = ps2.tile([1, NCH], F32, tag="row")
            nc.tensor.matmul(s_p[:], ones_col[:], t1x[:], start=True, stop=True)
            q_p = ps2.tile([1, NCH], F32, tag="row")
            nc.tensor.matmul(q_p[:], ones_colh[:], t1q[:], start=True, stop=True)
            m_row = a1.tile([1, NCH], F32R, tag=f"m_row{b}")
            nc.vector.tensor_scalar(m_row[:], s_p[:], 1.0 / D, None, ALU.mult)
            ms = a1.tile([1, NCH], F32, tag=f"ms{b}")
            nc.vector.tensor_mul(ms[:], m_row[:].bitcast(F32), m_row[:].bitcast(F32))
            var = a1.tile([1, NCH], F32, tag=f"var{b}")
            nc.vector.scalar_tensor_tensor(
                var[:], q_p[:], 1.0 / D, ms[:], ALU.mult, ALU.subtract
            )
            std_row = a1.tile([1, NCH], F32R, tag=f"std_row{b}")
            nc.scalar.activation(std_row[:], var[:], AF.Sqrt, bias=eps_col[:])
            r_row = a1.tile([1, NCH], F32, tag=f"r_row{b}")
            nc.vector.reciprocal_approx_fast(r_row[:], std_row[:].bitcast(F32))
            rr_row = a1.tile([1, NCH], F32R, tag=f"rr_row{b}")
            nc.vector.tensor_scalar(rr_row[:], r_row[:], 1.0, None, ALU.mult)
            return m_row, std_row, r_row, rr_row

        def rb_bcast(rr_row, b):
            p_rb = psb.tile([128, NCH], F32, tag="bc")
            nc.tensor.matmul(
                p_rb[:], ones_row[:], rr_row[:], start=True, stop=True
            )
            rb_s = a1.tile([128, NCH], F32, tag=f"rb_s{b}")
            nc.scalar.copy(rb_s[:], p_rb[:])
            return rb_s

        def partial_out(psum, gb_t, boff, mi, stage):
            """stage[:, mi, :] = psum + b/8 in fp16 (alternate engines)."""
            if mi % 2 == 0:
                with nc.allow_low_precision("fp16 allreduce payload"):
                    nc.vector.tensor_scalar(
                        stage[:, mi, :], psum[:, 0:T],
                        gb_t[:, boff + mi:boff + mi + 1], None, ALU.add
                    )
            else:
                nc.scalar.activation(
                    stage[:, mi, :], psum[:, 0:T], AF.Identity,
                    bias=gb_t[:, boff + mi:boff + mi + 1],
                )

        def all_reduce(l, s, b):
            if use_cc:
                nc.gpsimd.collective_compute(
                    "AllReduce", ALU.add,
                    replica_groups=[list(range(NC))],
                    ins=[ar_in[l, s, b][:].opt()],
                    outs=[ar_out[l, s, b][:].opt()],
                )
            else:
                nc.gpsimd.dma_start(out=ar_out[l, s, b][:], in_=ar_in[l, s, b][:])

        def refresh_xt(xb, l, s, b):
            tmp = a3.tile([128, KD, T], F16, tag=f"stage{b}")
            HK = KD // 2
            for q in range(2):
                k0, k1 = q * HK, (q + 1) * HK
                nc.sync.dma_start(
                    out=tmp[:, k0:k1, :], in_=ar_out[l, s, b][:, k0 * T:k1 * T]
                )
                nc.vector.tensor_add(
                    xb[:, k0:k1, 0:T], xb[:, k0:k1, 0:T].bitcast(F32),
                    tmp[:, k0:k1, :]
                )

        def xh_copy(xb, b):
            xh = a1.tile([128, KD, NCH], BF16, tag=f"xh{b}")
            HK = KD // 2
            nc.scalar.copy(xh[:, 0:HK, :], xb[:, 0:HK, :].bitcast(F32))
            nc.scalar.copy(xh[:, HK:KD, :], xb[:, HK:KD, :].bitcast(F32))
            return xh

        warm_in = nc.dram_tensor("warm_in", [128, 128], F16)
        warm_out = nc.dram_tensor("warm_out", [128, 128], F16, addr_space="Shared")


        def load_attn_weights(l):
            gb_t = cst.tile([128, GB_W], F32, tag="gb")
            nc.sync.dma_start(out=gb_t[:], in_=d_gb[l])
            wa = wgt.tile([128, KD, 256], F32R, tag="wa")
            nc.sync.dma_start(out=wa[:], in_=d_wattn[l])
            fat1 = cst.tile([1, 256], F32R, tag="fat1", bufs=1)
            nc.sync.dma_start(out=fat1[:], in_=d_fattn[l, 0:1, :])
            fat2 = cst.tile([1, 256], F32R, tag="fat2", bufs=1)
            nc.sync.dma_start(out=fat2[:], in_=d_fattn[l, 1:2, :])
            wp = wgt.tile([DH, HPC, D], BF16, tag="wp")
            nc.sync.dma_start(out=wp[:], in_=d_wproj[l])
            return gb_t, wa, fat1, fat2, wp

        def load_ffn_weights(l):
            wfc_t = wgt.tile([128, KD, FFL], WDT, tag="wfc")
            nc.sync.dma_start(out=wfc_t[:], in_=d_wfc[l])
            ffc1 = cst.tile([1, FFL], F32R, tag="ffc1", bufs=1)
            nc.sync.dma_start(out=ffc1[:], in_=d_ffc[l, 0:1, :])
            ffc2 = cst.tile([1, FFL], F32R, tag="ffc2", bufs=1)
            nc.sync.dma_start(out=ffc2[:], in_=d_ffc[l, 1:2, :])
            wout_t = wgt.tile([128, KF, D], WDT, tag="wout")
            nc.sync.dma_start(out=wout_t[:], in_=d_wout[l])
            return wfc_t, ffc1, ffc2, wout_t

        warm_sb = res.tile([128, 128], F16, tag="warm")
        nc.vector.memset(warm_sb[:], 0.0)
        nc.sync.dma_start(out=warm_in[:], in_=warm_sb[:])

        for _rep in range(reps):
            if use_cc:
                nc.gpsimd.collective_compute(
                    "AllReduce", ALU.add,
                    replica_groups=[list(range(NC))],
                    ins=[warm_in[:].opt()],
                    outs=[warm_out[:].opt()],
                )
            xtb = []
            for b in range(B):
                t = res.tile([128, KD, NCH], F32R, name=f"xt{b}", tag=f"xt{b}")
                nc.sync.dma_start(
                    out=t[:], in_=d_x0t[:, :, b * NCH:(b + 1) * NCH]
                )
                xtb.append(t)

            aw = load_attn_weights(0)
            fw = None
            aw_next = None
            for l in range(L):
                gb_t, wa, fat1, fat2, wp = aw

                # ---------- attention sublayer (per batch, pipelined) ------
                for b in range(B):
                    xb = xtb[b]
                    if l > 0:
                        refresh_xt(xb, l - 1, 1, b)
                    if b == 1:
                        fw = load_ffn_weights(l)
                    t1x, t1q = stats_trees(xb, b)

                    # raw Wg^T x groups, stats reduce interleaved
                    pw = []
                    for mi, (ms_, msz) in enumerate(MT):
                        p = ps3.tile([128, 256], F32, tag="mm")
                        for k in range(KD):
                            nc.tensor.matmul(
                                p[:msz, :], xb[:, k, ms_:ms_ + msz], wa[:, k, :],
                                start=(k == 0), stop=False,
                            )
                        pw.append(p)
                        if mi == 1:
                            m_row, std_row, r_row, rr_row = ln_rows(t1x, t1q, b)
                    # rank-1 fixups close each group; r as column per tile
                    rcol = a1.tile([128, 3], F32, tag=f"rcol{b}")
                    whsb = []
                    for mi, (ms_, msz) in enumerate(MT):
                        nc.tensor.matmul(
                            pw[mi][:msz, :],
                            m_row[:, ms_:ms_ + msz], fat1[:],
                            start=False, stop=False,
                        )
                        nc.tensor.matmul(
                            pw[mi][:msz, :],
                            std_row[:, ms_:ms_ + msz], fat2[:],
                            start=False, stop=True,
                        )
                        pt = ps2.tile([128, 1], F32, tag="row")
                        nc.tensor.transpose(
                            pt[:msz, :], r_row[:, ms_:ms_ + msz], ident[:1, :1]
                        )
                        nc.scalar.copy(rcol[:msz, mi:mi + 1], pt[:msz, :])
                        w = a1.tile([128, 198], F32R, tag=f"whsb{mi}_{b}")
                        nc.scalar.activation(
                            w[:msz, 0:196], pw[mi][:msz, 0:196], AF.Identity,
                            scale=rcol[:msz, mi:mi + 1],
                        )
                        nc.vector.tensor_scalar(
                            w[:msz, 196:198], w[:msz, 192:194].bitcast(F32),
                            0.2, None, ALU.mult
                        )
                        whsb.append(w)

                    erow = [
                        a1.tile([1, NCH], F32R, name=f"er{j}_{b}", tag=f"er{j}_{b}")
                        for j in range(HPC)
                    ]
                    for mi, (ms_, msz) in enumerate(MT):
                        for j in range(HPC):
                            pt = ps2.tile([1, 128], F32, tag="row")
                            nc.tensor.transpose(
                                pt[:, :msz],
                                whsb[mi][:msz, 194 + j:195 + j].bitcast(F32),
                                ident[:msz, :msz],
                            )
                            nc.scalar.copy(erow[j][:, ms_:ms_ + msz], pt[:, :msz])

                    aggt = []
                    for j in range(HPC):
                        p_er = psb.tile([128, NCH], F32, tag="bc")
                        nc.tensor.matmul(
                            p_er[:], ones_row[:], erow[j][:], start=True, stop=True
                        )
                        e_tiles = []
                        for mi in range(3):
                            rsz = MT_REAL[mi]
                            e1 = a2.tile([128, NCH], F32R, tag=f"e{mi}_{b}", bufs=1)
                            nc.scalar.activation(
                                e1[:rsz, :], p_er[:rsz, :], AF.Exp,
                                bias=whsb[mi][:rsz, 192 + j:193 + j].bitcast(F32),
                            )
                            e2 = a1.tile([128, NCH], F32, tag="e2")
                            nc.scalar.activation(
                                e2[:rsz, :], p_er[:rsz, :], AF.Exp, scale=0.2,
                                bias=whsb[mi][:rsz, 196 + j:197 + j].bitcast(F32),
                            )
                            nc.vector.tensor_max(
                                e1[:rsz, :], e1[:rsz, :].bitcast(F32), e2[:rsz, :]
                            )
                            e_tiles.append(e1)
                        p_s = ps2.tile([1, NCH], F32, tag="row")
                        for mi in range(3):
                            rsz = MT_REAL[mi]
                            nc.tensor.matmul(
                                p_s[:], ones_col[:rsz, :], e_tiles[mi][:rsz, :],
                                start=(mi == 0), stop=(mi == 2),
                            )
                        sr = a1.tile([1, NCH], F32, tag=f"sr{b}")
                        nc.vector.reciprocal_approx_fast(sr[:], p_s[:])
                        srr = a1.tile([1, NCH], F32R, tag=f"srr{b}")
                        nc.vector.tensor_scalar(srr[:], sr[:], 1.0, None, ALU.mult)
                        p_rb2 = psb.tile([DH, NCH], F32, tag="bc")
                        nc.tensor.matmul(
                            p_rb2[:], ones_row[:, :DH], srr[:],
                            start=True, stop=True,
                        )
                        rb_sb = a1.tile([DH, NCH], F32, tag=f"rb_sb{b}")
                        nc.scalar.copy(rb_sb[:], p_rb2[:])
                        p_agg = ps3.tile([DH, NCH], F32, tag="mm")
                        for mi in range(3):
                            rsz = MT_REAL[mi]
                            nc.tensor.matmul(
                                p_agg[:],
                                whsb[mi][:rsz, j * DH:(j + 1) * DH],
                                e_tiles[mi][:rsz, :],
                                start=(mi == 0), stop=(mi == 2),
                            )
                        at = a1.tile([DH, NCH], BF16, tag=f"aggt{j}_{b}")
                        with nc.allow_low_precision("bf16 agg"):
                            nc.vector.tensor_mul(at[:], p_agg[:], rb_sb[:])
                        aggt.append(at)

                    stage = a3.tile([128, KD, T], F16, tag=f"stage{b}")
                    for mi in range(KD):
                        p = ps3.tile([128, NCH], F32, tag="mm")
                        for j in range(HPC):
                            nc.tensor.matmul(
                                p[:], wp[:, j, mi * 128:(mi + 1) * 128], aggt[j][:],
                                start=(j == 0), stop=(j == HPC - 1),
                            )
                        partial_out(p, gb_t, GB_BPROJ, mi, stage)
                    nc.sync.dma_start(out=ar_in[l, 0, b][:], in_=stage[:])
                    all_reduce(l, 0, b)

                # ---------- FFN sublayer (per batch, pipelined) ------------
                wfc_t, ffc1, ffc2, wout_t = fw

                for b in range(B):
                    xb = xtb[b]
                    refresh_xt(xb, l, 0, b)
                    if b == 1 and l < L - 1:
                        aw_next = load_attn_weights(l + 1)
                    if b == 1 and l == L - 1:
                        fhd1 = cst.tile([1, VL], F32R, tag="fhd1", bufs=1)
                        nc.sync.dma_start(out=fhd1[:], in_=d_fhead[0:1, :])
                        fhd2 = cst.tile([1, VL], F32R, tag="fhd2", bufs=1)
                        nc.sync.dma_start(out=fhd2[:], in_=d_fhead[1:2, :])
                        VLQ = VL // 4
                        wh_pre = wgt.tile([128, KD, VLQ], BF16, tag="whd0")
                        nc.sync.dma_start(
                            out=wh_pre[:], in_=d_whead[:, :, 0:VLQ]
                        )
                    xh = xh_copy(xb, b)
                    t1x, t1q = stats_trees(xb, b)

                    g_tiles = [None] * KF
                    open_p = {}

                    def fc_open(mi):
                        p = ps3.tile([128, NCH], F32, tag="mm")
                        for k in range(KD):
                            nc.tensor.matmul(
                                p[:], wfc_t[:, k, mi * 128:(mi + 1) * 128],
                                xh[:, k, :],
                                start=(k == 0), stop=False,
                            )
                        open_p[mi] = p

                    def fc_close(mi, rb_s):
                        p = open_p.pop(mi)
                        nc.tensor.matmul(
                            p[:], ffc1[:, mi * 128:(mi + 1) * 128],
                            m_row[:],
                            start=False, stop=False,
                        )
                        nc.tensor.matmul(
                            p[:], ffc2[:, mi * 128:(mi + 1) * 128],
                            std_row[:],
                            start=False, stop=True,
                        )
                        gf_ = a2.tile([128, NCH], F32, tag="gf")
                        nc.vector.tensor_mul(gf_[:], p[:], rb_s[:])
                        g = a2.tile([128, NCH], WDT, tag=f"g{mi}_{b}", bufs=1)
                        nc.scalar.activation(
                            g[:], gf_[:], AF.Gelu,
                            bias=gb_t[:, GB_BFC + mi:GB_BFC + mi + 1],
                        )
                        g_tiles[mi] = g

                    fc_open(0)
                    fc_open(1)
                    m_row, std_row, r_row, rr_row = ln_rows(t1x, t1q, b)
                    fc_open(2)
                    fc_open(3)
                    rb_s = rb_bcast(rr_row, b)
                    fc_close(0, rb_s)
                    fc_open(4)
                    fc_close(1, rb_s)
                    fc_open(5)
                    fc_close(2, rb_s)
                    fc_close(3, rb_s)
                    fc_close(4, rb_s)
                    fc_close(5, rb_s)

                    stage = a3.tile([128, KD, T], F16, tag=f"stage{b}")
                    for mi in range(KD):
                        p = ps3.tile([128, NCH], F32, tag="mm")
                        for k in range(KF):
                            nc.tensor.matmul(
                                p[:], wout_t[:, k, mi * 128:(mi + 1) * 128],
                                g_tiles[k][:],
                                start=(k == 0), stop=(k == KF - 1),
                            )
                        partial_out(p, gb_t, GB_BOUT, mi, stage)
                    nc.sync.dma_start(out=ar_in[l, 1, b][:], in_=stage[:])
                    all_reduce(l, 1, b)
                if l < L - 1:
                    aw = aw_next

            # ---------- final LN + vocab-sharded head ----------
            KVQ = KV // 4

            def head_mms(wh_t, xh, m_row, std_row, rb_s, b, quarter):
                for mi in range(KVQ):
                    gmi = quarter * KVQ + mi
                    p = ps3.tile([128, NCH], F32, tag="mm")
                    for k in range(KD):
                        nc.tensor.matmul(
                            p[:], wh_t[:, k, mi * 128:(mi + 1) * 128],
                            xh[:, k, :],
                            start=(k == 0), stop=False,
                        )
                    nc.tensor.matmul(
                        p[:], fhd1[:, gmi * 128:(gmi + 1) * 128],
                        m_row[:],
                        start=False, stop=False,
                    )
                    nc.tensor.matmul(
                        p[:], fhd2[:, gmi * 128:(gmi + 1) * 128],
                        std_row[:],
                        start=False, stop=True,
                    )
                    lg = a2.tile([128, NCH], F32, tag=f"part1{b}", bufs=1)
                    nc.vector.tensor_mul(lg[:], p[:], rb_s[:])
                    nc.sync.dma_start(
                        out=d_logits[gmi * 128:(gmi + 1) * 128, b * T:(b + 1) * T],
                        in_=lg[:, 0:T],
                    )

            qi = 0
            for b in range(B):
                refresh_xt(xtb[b], L - 1, 1, b)
                xh = xh_copy(xtb[b], b)
                t1x, t1q = stats_trees(xtb[b], b)
                m_row, std_row, r_row, rr_row = ln_rows(t1x, t1q, b)
                rb_s = rb_bcast(rr_row, b)
                for quarter in range(4):
                    if qi == 0:
                        wh_t = wh_pre
                    else:
                        wh_t = wgt.tile(
                            [128, KD, VLQ], BF16, tag=f"whd{qi % 2}"
                        )
                        nc.sync.dma_start(
                            out=wh_t[:],
                            in_=d_whead[:, :, quarter * VLQ:(quarter + 1) * VLQ],
                        )
                    head_mms(wh_t, xh, m_row, std_row, rb_s, b, quarter)
                    qi += 1

    nc.compile()
    return nc


def _get_nc(reps=1, use_cc=True, ffn_bf16=True):
    key = f"nc{reps}_{use_cc}_{ffn_bf16}"
    if key not in _CACHE:
        _CACHE[key] = _build_nc(reps, use_cc, ffn_bf16)
    return _CACHE[key]


# --------------------------------------------------------------------------
# numpy fallback (exact reference semantics for arbitrary edges)
# --------------------------------------------------------------------------

def _numpy_forward(inp):
    from scipy.special import erf

    def ln(x, g, b):
        m = x.mean(-1, keepdims=True)
        v = ((x - m) ** 2).mean(-1, keepdims=True)
        return (x - m) / np.sqrt(v + EPS) * g + b

    f32 = np.float32
    objs_e = np.asarray(inp["obj_emb_w"])[np.asarray(inp["objs"])]
    pe = np.asarray(inp["poss_emb_w"])[np.asarray(inp["poss"])]
    nfeat = np.concatenate([objs_e, pe[:, :NOBJ], pe[:, NOBJ:]], axis=-1)
    z = np.asarray(inp["tok_emb"])[np.asarray(inp["z_indices"])]
    x = np.concatenate([nfeat, z], axis=1) + np.asarray(inp["pos_emb"])[:, :T]
    x = x.reshape(N, D).astype(f32)
    src = np.asarray(inp["src"]).astype(np.int64)
    dst = np.asarray(inp["dst"]).astype(np.int64)
    for l in range(L):
        h = ln(x, inp["ln1_g"][l], inp["ln1_b"][l])
        Wh = (h @ np.asarray(inp["W_attn"][l])).reshape(N, H, DH)
        el = np.einsum("nhd,hd->nh", Wh, np.asarray(inp["a_l"][l]))
        er = np.einsum("nhd,hd->nh", Wh, np.asarray(inp["a_r"][l]))
        e = el[src] + er[dst]
        e = np.where(e >= 0, e, 0.2 * e)
        m = np.full((N, H), -np.inf, f32)
        np.maximum.at(m, dst, e)
        m[~np.isfinite(m)] = 0.0
        ex = np.exp(e - m[dst])
        s = np.zeros((N, H), f32)
        np.add.at(s, dst, ex)
        alpha = ex / s[dst]
        agg = np.zeros((N, H, DH), f32)
        np.add.at(agg, dst, alpha[:, :, None] * Wh[src])
        x = x + agg.reshape(N, D) @ np.asarray(inp["W_proj"][l]) \
            + np.asarray(inp["b_proj"][l])
        h2 = ln(x, inp["ln2_g"][l], inp["ln2_b"][l])
        ff = h2 @ np.asarray(inp["W_fc"][l]) + np.asarray(inp["b_fc"][l])
        ff = ff * 0.5 * (1.0 + erf(ff / np.sqrt(2.0)))
        x = x + ff @ np.asarray(inp["W_out"][l]) + np.asarray(inp["b_out"][l])
    x = ln(x, inp["lnf_g"], inp["lnf_b"])
    return (x @ np.asarray(inp["head_w"])).reshape(B, T, V).astype(f32)


# --------------------------------------------------------------------------
# public entry
# --------------------------------------------------------------------------

def _edges_are_block_diag(inp):
    src, dst = _block_diag_edges_np()
    s = np.asarray(inp["src"])
    d = np.asarray(inp["dst"])
    return (
        s.shape == src.shape
        and np.array_equal(s.astype(np.int64), src)
        and np.array_equal(d.astype(np.int64), dst)
    )


def _assemble(results):
    full = np.concatenate([results[c]["logits"] for c in range(NC)], axis=0)
    return np.ascontiguousarray(full.T).reshape(B, T, V)


def kernel(**inputs):
    if not _edges_are_block_diag(inputs):
        return _numpy_forward(inputs)
    from concourse import bass2jax

    in_maps = _host_inputs(inputs)
    results = bass2jax.run_bass_via_pjrt(_get_nc(), in_maps, n_cores=NC)
    return _assemble(results)


# --------------------------------------------------------------------------
# benchmarking (repeated execution, device-resident inputs)
# --------------------------------------------------------------------------

def _make_runner(nc):
    """Persistent jitted shard_map callable for nc (multi-core), mirroring
    bass2jax.run_bass_via_pjrt but reusable across calls."""
    import jax
    from jax.sharding import Mesh, PartitionSpec
    from jax.experimental.shard_map import shard_map
    from concourse import bass2jax, mybir as _mybir

    bass2jax.install_neuronx_cc_hook()
    partition_name = nc.partition_id_tensor.name if nc.partition_id_tensor else None
    in_names, out_names, out_avals, zero_outs = [], [], [], []
    for alloc in nc.m.functions[0].allocations:
        if not isinstance(alloc, _mybir.MemoryLocationSet):
            continue
        name = alloc.memorylocations[0].name
        if alloc.kind == "ExternalInput":
            if name != partition_name:
                in_names.append(name)
        elif alloc.kind == "ExternalOutput":
            shape = tuple(alloc.tensor_shape)
            dtype = _mybir.dt.np(alloc.dtype)
            out_names.append(name)
            out_avals.append(jax.core.ShapedArray(shape, dtype))
            zero_outs.append(np.zeros(shape, dtype))
    n_params = len(in_names)
    all_in_names = list(in_names) + list(out_names)
    if partition_name is not None:
        all_in_names.append(partition_name)

    def _body(*args):
        operands = list(args)
        if partition_name is not None:
            operands.append(bass2jax.partition_id_tensor())
        return tuple(
            bass2jax._bass_exec_p.bind(
                *operands,
                out_avals=tuple(out_avals),
                in_names=tuple(all_in_names),
                out_names=tuple(out_names),
                lowering_input_output_aliases=(),
                sim_require_finite=True,
                sim_require_nnan=True,
                nc=nc,
            )
        )

    devices = jax.devices()[:NC]
    mesh = Mesh(np.asarray(devices), ("core",))
    n_outs = len(out_names)
    in_specs = (PartitionSpec("core"),) * (n_params + n_outs)
    out_specs = (PartitionSpec("core"),) * n_outs
    donate = tuple(range(n_params, n_params + n_outs))
    fn = jax.jit(
        shard_map(_body, mesh=mesh, in_specs=in_specs, out_specs=out_specs,
                  check_rep=False),
        donate_argnums=donate, keep_unused=True,
    )
    return fn, in_names, out_names, zero_outs, mesh


def _timed_run(nc, in_maps, iters):
    """Median wall time (s) per execution with device-resident inputs."""
    import jax

    from jax.sharding import NamedSharding, PartitionSpec

    fn, in_names, out_names, zero_outs, mesh = _make_runner(nc)
    shard = NamedSharding(mesh, PartitionSpec("core"))
    concat_in = [
        np.concatenate([np.asarray(m[name]) for m in in_maps], axis=0)
        for name in in_names
    ]
    dev_in = [jax.device_put(a, shard) for a in concat_in]
    jax.block_until_ready(dev_in)

    def zeros():
        zs = [
            jax.device_put(
                np.zeros((NC * z.shape[0], *z.shape[1:]), z.dtype), shard
            )
            for z in zero_outs
        ]
        jax.block_until_ready(zs)
        return zs

    outs = fn(*dev_in, *zeros())  # warm-up/compile
    jax.block_until_ready(outs)
    times = []
    for _ in range(iters):
        zs = zeros()
        t0 = time.perf_counter()
        outs = fn(*dev_in, *zs)
        jax.block_until_ready(outs)
        times.append(time.perf_counter() - t0)
    return float(np.min(times)), outs, out_names


def bench(inputs, iters=16):
    """HW ns per network pass via reps-differential (cancels dispatch cost)."""
    in_maps = _host_inputs(inputs)
    t1, _, _ = _timed_run(_get_nc(1), in_maps, iters)
    t9, _, _ = _timed_run(_get_nc(9), in_maps, iters)
    print(f"  wall/iter reps1: {t1 * 1e6:.0f} us,  reps9: {t9 * 1e6:.0f} us")
    return max(t9 - t1, 0.0) / 8 * 1e9
